# revision 25
# baseline (speedup 1.0000x reference)
"""GCN actor kernel for 8 TRN2 NeuronCores (Bass/Tile).

Math (mirrors the reference):
    deg[v]  = in-degree(v) + 1 (self loop);  dinv = deg^-1/2
    y[v]    = dinv[v] * x[v]                         (folded on host)
    acc[v]  = sum_{(s,v) in E} y[s] + y[v]           (segment sum + self loop)
    h[v]    = relu(dinv[v] * (acc[v] @ conv_w) + conv_b)
    z[v]    = (h[v] - mean) * rsqrt(var + eps)       (LayerNorm core)
    pooled  = allreduce(sum_v z[v]) * ln_g + N * ln_b
    out     = tanh(relu(pooled @ w2 + b2) @ w3 + b3)

Sharding: nodes (and their incoming edges) are dst-sharded across the 8
cores; the y table (bf16, split xa/xb at 32768 rows for int16 SWDGE
idxs) lives replicated in DRAM and per-edge messages are fetched with
dma_gather.  Only the [128] pooled vector is all-reduced.

Aggregation runs in the FLIPPED orientation: each matmul computes
accT[feat, dstcols] += gather_tile[tok, feat]^T @ S[tok, dstcols] where
S is a one-hot built on DVE (iota == dstrel, batched per chunk via
stride-0 broadcast APs).  Node tiles are processed in GROUPS of two
(256 dst columns, one PSUM region).  Per 64-dst window, full 128-token
tiles use 64-wide S; the leftover tokens of a group's four windows are
merged into 256-wide "wide" tiles, so window round-up padding is tiny.
Self-loop contributions are not gathered at all: the core's own y rows
are streamed sequentially as a transposed [feat, NPC] slice and added
during PSUM eviction.  Tiling counts are made uniform across cores
(min/max over cores; pad tokens carry dstrel=-1 and match nothing) so
the SPMD graph is identical; only idx/dstrel DATA differs per core.

SWDGE idx layout: idx i of an instruction lives at [i % 16, i // 16] in
an int16 SBUF tile, rows 0-15 replicated into 16-31.  Gathers are
issued 1024 tokens at a time (the SWDGE idx-streamer cap).
"""

import numpy as np
import ml_dtypes

import concourse.bass as bass
import concourse.bacc as bacc
import concourse.tile as tile
import concourse.mybir as mybir
from concourse.bass_utils import run_bass_kernel_spmd
from concourse.masks import make_identity

F32 = mybir.dt.float32
BF16 = mybir.dt.bfloat16
I16 = mybir.dt.int16
NPBF = ml_dtypes.bfloat16

NCORES = 8
D = 128          # feature dim (D_IN == D_H == 128)
DA = 64          # action dim
LN_EPS = 1e-5
W = 64           # dst window for full tiles
GCOL = 256       # dst columns per node-tile group (2 tiles of 128)
CHT = 8          # tiles per gather chunk (8*128 = 1024 token SWDGE cap)
HALF = 32768     # xa/xb table split (int16 idx range)


def _round_up(a, b):
    return -(-a // b) * b


def build_graph(cfg):
    """Build + compile the SPMD Bass graph. cfg keys:
    NPAD, fullw_a/fullw_b (tokens-per-window tile counts, uniform),
    nwide_a/nwide_b (wide tiles per group), debug, single."""
    NPAD = cfg["NPAD"]
    STAGE = cfg.get("stage", 99)
    NPC = NPAD // NCORES
    NTO = NPC // 128
    NG = (NTO + 1) // 2

    # processing-order tile lists per stream: (group, kind, ncols, coloff)
    # kind: 0 full (ncols=W, coloff within group), 1 wide (ncols=GCOL)
    streams = []
    for s, (fullw, nwide) in enumerate(
        [(cfg["fullw_a"], cfg["nwide_a"]), (cfg["fullw_b"], cfg["nwide_b"])]
    ):
        tiles = []          # per tile: (group, coloff, ncols, kind_idx)
        nf = nw = 0
        for g in range(NG):
            for wi in range(4):
                wabs = g * 4 + wi
                if wabs >= len(fullw):
                    continue
                for _ in range(fullw[wabs]):
                    tiles.append((g, wi * W, W, nf))
                    nf += 1
            for _ in range(nwide[g]):
                tiles.append((g, 0, GCOL, nw))
                nw += 1
        wcnt = list(nwide)
        wbase = [0] * len(wcnt)
        for g in range(1, len(wcnt)):
            wbase[g] = wbase[g - 1] + wcnt[g - 1]
        streams.append(
            {"tiles": tiles, "nf": nf, "nw": nw, "wcnt": wcnt, "wbase": wbase}
        )

    nc = bacc.Bacc(
        "TRN2",
        target_bir_lowering=False,
        debug=cfg.get("debug", False),
        num_devices=NCORES,
        dynamic_dma_scratch_size=32768,
    )

    xa = nc.dram_tensor("xa", [min(HALF, NPAD), D], BF16, kind="ExternalInput")
    if NPAD > HALF:
        xb = nc.dram_tensor("xb", [NPAD - HALF, D], BF16, kind="ExternalInput")
    cw = nc.dram_tensor("cw", [D, D], BF16, kind="ExternalInput")
    exts = {}
    for s in range(2):
        ntiles = len(streams[s]["tiles"])
        if not ntiles:
            continue
        exts[f"src{s}"] = nc.dram_tensor(
            f"src{s}", [128, ntiles * 8], I16, kind="ExternalInput"
        )
        if streams[s]["nf"]:
            exts[f"drf{s}"] = nc.dram_tensor(
                f"drf{s}", [128, streams[s]["nf"]], BF16, kind="ExternalInput"
            )
        if streams[s]["nw"]:
            exts[f"drw{s}"] = nc.dram_tensor(
                f"drw{s}", [128, streams[s]["nw"]], BF16, kind="ExternalInput"
            )
    iwf = nc.dram_tensor("iwf", [W], BF16, kind="ExternalInput")
    iww = nc.dram_tensor("iww", [GCOL], BF16, kind="ExternalInput")
    yselfT = nc.dram_tensor("yselfT", [128, NPC], BF16, kind="ExternalInput")
    dinvo = nc.dram_tensor("dinvo", [128, NTO], F32, kind="ExternalInput")
    cb = nc.dram_tensor("cb", [D], F32, kind="ExternalInput")
    w2 = nc.dram_tensor("w2", [D, D], F32, kind="ExternalInput")
    b2 = nc.dram_tensor("b2", [D], F32, kind="ExternalInput")
    w3 = nc.dram_tensor("w3", [D, DA], F32, kind="ExternalInput")
    b3 = nc.dram_tensor("b3", [DA], F32, kind="ExternalInput")
    out_ext = nc.dram_tensor("out", [DA, 1], F32, kind="ExternalOutput")

    cc_in = nc.dram_tensor("cc_in", [D, 1], F32)
    cc_out = nc.dram_tensor("cc_out", [D, 1], F32, addr_space="Shared")

    with tile.TileContext(nc) as tc:
        with tc.tile_pool(name="persist", bufs=1) as per:
            # HW only reads idx partitions 0-31 (RX/TX Q7 cores); CoreSim
            # asserts all 128 rows, so load the full tile only in debug
            IDXR = 128 if (cfg.get("debug") or cfg.get("fullidx")) else 32
            st = []
            for s in range(2):
                ntiles = len(streams[s]["tiles"])
                d = dict(streams[s])
                if ntiles:
                    d["it"] = per.tile([128, ntiles * 8], I16, name=f"it{s}")
                    hc = min(CHT, ntiles) * 8
                    nc.sync.dma_start(
                        out=d["it"][:IDXR, :hc], in_=exts[f"src{s}"][:IDXR, :hc]
                    )
                    if ntiles * 8 > hc:
                        nc.sync.dma_start(
                            out=d["it"][:IDXR, hc:], in_=exts[f"src{s}"][:IDXR, hc:]
                        )
                    if d["nf"]:
                        d["drf"] = per.tile([128, d["nf"]], BF16, name=f"drf{s}t")
                        nc.sync.dma_start(out=d["drf"][:], in_=exts[f"drf{s}"][:, :])
                    if d["nw"]:
                        d["drw"] = per.tile([128, d["nw"]], BF16, name=f"drw{s}t")
                        nc.sync.dma_start(out=d["drw"][:], in_=exts[f"drw{s}"][:, :])
                    d["src"] = (xa if s == 0 else xb)[:, :]
                d["chunks"] = {}
                st.append(d)
            iwf_t = per.tile([128, W], BF16)
            nc.sync.dma_start(
                out=iwf_t[:], in_=bass.AP(tensor=iwf, offset=0, ap=[[0, 128], [1, W]])
            )
            iww_t = per.tile([128, GCOL], BF16)
            nc.sync.dma_start(
                out=iww_t[:], in_=bass.AP(tensor=iww, offset=0, ap=[[0, 128], [1, GCOL]])
            )

            NG_local = NG
            # group -> list of (stream, tile_index) in processing order
            tiles_by_group = [[] for _ in range(NG_local)]
            for s in range(2):
                for t, (g, coloff, ncols, ki) in enumerate(st[s]["tiles"]):
                    tiles_by_group[g].append((s, t))

            ntok_regs = {}

            with (
                tc.tile_pool(name="gbuf", bufs=10) as gbp,
                tc.tile_pool(name="sfull", bufs=8) as sfp,
                tc.tile_pool(name="swide", bufs=10) as swp,
                tc.tile_pool(name="bpg", bufs=4, space="PSUM") as bpg,
                tc.tile_pool(name="bph", bufs=2, space="PSUM") as bph,
                tc.tile_pool(name="chh", bufs=3) as chh,
                tc.tile_pool(name="cst", bufs=6) as cst,
                tc.tile_pool(name="cps", bufs=1, space="PSUM") as cps,
            ):

                def ensure_chunk(s, c):
                    S = st[s]
                    if c in S["chunks"]:
                        return S["chunks"][c]
                    ntiles = len(S["tiles"])
                    ntc = min(CHT, ntiles - c * CHT)
                    ntok = ntc * 128
                    if ntok not in ntok_regs:
                        ntok_regs[ntok] = nc.gpsimd.to_reg(ntok)
                    g = gbp.tile([128, ntc, D], BF16, tag=f"g{s}")
                    nc.gpsimd.dma_gather(
                        g[:, :, :],
                        S["src"],
                        S["it"][:, c * CHT * 8 : (c * CHT + ntc) * 8],
                        ntok,
                        ntok_regs[ntok],
                        D,
                    )
                    # batched one-hot S builds: full tiles and wide tiles of
                    # this chunk each occupy consecutive drf/drw columns
                    ch_tiles = S["tiles"][c * CHT : c * CHT + ntc]
                    fulls = [x for x in ch_tiles if x[2] == W]
                    sm_f = None
                    if fulls and STAGE >= 2:
                        nf0 = fulls[0][3]
                        nf = len(fulls)
                        sm_f = sfp.tile([128, nf, W], BF16, tag=f"sf{s}")
                        dsl = S["drf"][:, nf0 : nf0 + nf]
                        nc.vector.tensor_tensor(
                            out=sm_f[:, :, :],
                            in0=bass.AP(
                                tensor=iwf_t.tensor,
                                offset=iwf_t.offset,
                                ap=[iwf_t[:].ap[0], [0, nf], [1, W]],
                            ),
                            in1=bass.AP(
                                tensor=dsl.tensor,
                                offset=dsl.offset,
                                ap=[dsl.ap[0], dsl.ap[1], [0, W]],
                            ),
                            op=mybir.AluOpType.is_equal,
                        )
                        sm_f = (sm_f, nf0)
                    S["chunks"][c] = (g, sm_f)
                    return S["chunks"][c]

                WB = 8

                def ensure_wides(s, g0):
                    S = st[s]
                    key = ("w", g0)
                    if key in S["chunks"]:
                        return S["chunks"][key]
                    nw0 = S["wbase"][g0]
                    nw = S["wcnt"][g0]
                    subs = []
                    for o in range(0, nw, WB):
                        n = min(WB, nw - o)
                        sm_w = swp.tile([128, n, GCOL], BF16, tag=f"sw{s}")
                        dsl = S["drw"][:, nw0 + o : nw0 + o + n]
                        nc.vector.tensor_tensor(
                            out=sm_w[:, :, :],
                            in0=bass.AP(
                                tensor=iww_t.tensor,
                                offset=iww_t.offset,
                                ap=[iww_t[:].ap[0], [0, n], [1, GCOL]],
                            ),
                            in1=bass.AP(
                                tensor=dsl.tensor,
                                offset=dsl.offset,
                                ap=[dsl.ap[0], dsl.ap[1], [0, GCOL]],
                            ),
                            op=mybir.AluOpType.is_equal,
                        )
                        subs.append((sm_w, nw0 + o, n))
                    S["chunks"][key] = subs
                    return S["chunks"][key]

                # prefetch first chunks so gathers start before bulk preamble
                for s in range(2):
                    if st[s]["tiles"]:
                        ensure_chunk(s, 0)

                cw_t = per.tile([D, D], BF16)
                nc.sync.dma_start(out=cw_t[:], in_=cw[:, :])
                ident = per.tile([128, 128], F32)
                make_identity(nc, ident[:])
                ys_t = per.tile([128, NPC], BF16)
                nc.sync.dma_start(out=ys_t[:], in_=yselfT[:, :])
                dinvo_t = per.tile([128, NTO], F32)
                nc.sync.dma_start(out=dinvo_t[:], in_=dinvo[:, :])
                cb_t = per.tile([128, D], F32)
                nc.sync.dma_start(
                    out=cb_t[:], in_=bass.AP(tensor=cb, offset=0, ap=[[0, 128], [1, D]])
                )
                eps_t = per.tile([128, 1], F32)
                nc.vector.memset(eps_t[:], LN_EPS)
                pool_t = per.tile([128, D], F32)
                nc.vector.memset(pool_t[:], 0.0)
                # tail weights, preloaded so the final MLP has no DMA waits
                w2_t = per.tile([D, D], F32)
                nc.sync.dma_start(out=w2_t[:], in_=w2[:, :])
                b2_t = per.tile([D, 1], F32)
                nc.sync.dma_start(out=b2_t[:], in_=b2[:, None])
                w3_t = per.tile([D, DA], F32)
                nc.sync.dma_start(out=w3_t[:], in_=w3[:, :])
                b3_t = per.tile([DA, 1], F32)
                nc.sync.dma_start(out=b3_t[:], in_=b3[:, None])
                # activation-table warmup: touch Relu/Sqrt/Tanh during the bulk
                # phase so the ~1.3us table loads don't land in the tail
                warm = per.tile([128, 1], F32)
                nc.scalar.activation(
                    out=warm[:], in_=eps_t[:], func=mybir.ActivationFunctionType.Relu
                )
                nc.scalar.activation(
                    out=warm[:], in_=eps_t[:], func=mybir.ActivationFunctionType.Sqrt
                )
                nc.scalar.activation(
                    out=warm[:], in_=eps_t[:], func=mybir.ActivationFunctionType.Tanh
                )

                for g in range(NG_local):
                    ntiles_g = min(2, NTO - g * 2)  # 2, or 1 for phantom group
                    for s in range(2):
                        for ga in (g, g + 1, g + 2):
                            if (
                                ga < NG_local
                                and st[s].get("wcnt")
                                and st[s]["wcnt"][ga]
                            ):
                                ensure_wides(s, ga)
                    if STAGE < 3:
                        for (s, t) in tiles_by_group[g]:
                            ensure_chunk(s, t // CHT)
                            if STAGE >= 2 and st[s]["tiles"][t][2] == GCOL:
                                ensure_wides(s, g)
                        continue
                    acc = bpg.tile([128, GCOL], F32, tag="acc")
                    nc.vector.memset(acc[:], 0.0)
                    mms = tiles_by_group[g]
                    for i, (s, t) in enumerate(mms):
                        gt, sm_f = ensure_chunk(s, t // CHT)
                        k = t % CHT
                        _, coloff, ncols, ki = st[s]["tiles"][t]
                        if ncols == W:
                            smt, base = sm_f
                            rhs = smt[:, ki - base, :]
                        else:
                            for smt, base, n in ensure_wides(s, g):
                                if base <= ki < base + n:
                                    rhs = smt[:, ki - base, :]
                                    break
                        nc.tensor.matmul(
                            acc[:, coloff : coloff + ncols],
                            lhsT=gt[:, k, :],
                            rhs=rhs,
                            start=False,
                            stop=(i == len(mms) - 1),
                            skip_group_check=True,
                        )
                    if STAGE < 4:
                        continue
                    # evict + self-loop add: accT_sb = acc + yselfT cols
                    ev = chh.tile([128, GCOL], BF16, tag="ev")
                    nc.vector.tensor_add(
                        out=ev[:, : ntiles_g * 128],
                        in0=acc[:, : ntiles_g * 128],
                        in1=ys_t[:, g * 256 : g * 256 + ntiles_g * 128],
                    )
                    if STAGE < 5:
                        continue
                    for jj in range(ntiles_g):
                        j = g * 2 + jj
                        hps = bph.tile([128, D], F32)
                        nc.tensor.matmul(
                            hps[:],
                            lhsT=ev[:, jj * 128 : (jj + 1) * 128],
                            rhs=cw_t[:],
                            start=True,
                            stop=True,
                        )
                        if STAGE < 6:
                            continue
                        # epilogue: h = relu(dinv * hw + cb); LN core; z-pool
                        h = chh.tile([128, D], F32, tag="h")
                        nc.vector.scalar_tensor_tensor(
                            out=h[:],
                            in0=hps[:],
                            scalar=dinvo_t[:, j : j + 1],
                            in1=cb_t[:],
                            op0=mybir.AluOpType.mult,
                            op1=mybir.AluOpType.add,
                        )
                        nc.scalar.activation(
                            out=h[:], in_=h[:], func=mybir.ActivationFunctionType.Relu
                        )
                        stt = cst.tile([128, nc.vector.BN_STATS_DIM], F32)
                        nc.vector.bn_stats(out=stt[:], in_=h[:])
                        mv = cst.tile([128, nc.vector.BN_AGGR_DIM], F32)
                        nc.vector.bn_aggr(out=mv[:], in_=stt[:])
                        sq = cst.tile([128, 1], F32)
                        nc.scalar.activation(
                            out=sq[:],
                            in_=mv[:, 1:2],
                            func=mybir.ActivationFunctionType.Sqrt,
                            bias=eps_t[:],
                        )
                        rstd = cst.tile([128, 1], F32)
                        nc.vector.reciprocal(out=rstd[:], in_=sq[:])
                        nc.vector.tensor_scalar(
                            out=h[:],
                            in0=h[:],
                            scalar1=mv[:, 0:1],
                            scalar2=rstd[:],
                            op0=mybir.AluOpType.subtract,
                            op1=mybir.AluOpType.mult,
                        )
                        nc.vector.tensor_add(out=pool_t[:], in0=pool_t[:], in1=h[:])

                # transpose pool_t -> [feature, 1] column
                pps = cps.tile([128, 128], F32, tag="tail")
                nc.tensor.transpose(out=pps[:], in_=pool_t[:], identity=ident[:])
                tp = chh.tile([128, 128], F32, tag="tp")
                nc.vector.tensor_copy(out=tp[:], in_=pps[:])
                zsum = per.tile([128, 1], F32)
                nc.vector.tensor_reduce(
                    out=zsum[:],
                    in_=tp[:],
                    axis=mybir.AxisListType.X,
                    op=mybir.AluOpType.add,
                )

                # ---------------- phase D: all-reduce + MLP ----------------
                nc.scalar.activation(
                    out=warm[:], in_=eps_t[:], func=mybir.ActivationFunctionType.Tanh
                )
                nc.sync.dma_start(out=cc_in[:, :], in_=zsum[:])
                if cfg.get("single"):
                    nc.sync.dma_start(out=cc_out[:, :], in_=cc_in[:, :])
                else:
                    nc.gpsimd.collective_compute(
                        "AllReduce",
                        mybir.AluOpType.add,
                        replica_groups=[list(range(NCORES))],
                        ins=[cc_in.ap().opt()],
                        outs=[cc_out.ap().opt()],
                    )
                pooled = per.tile([128, 1], F32)
                nc.sync.dma_start(out=pooled[:], in_=cc_out[:, :])

                ps2 = cps.tile([D, 1], F32, tag="tail")
                nc.tensor.matmul(ps2[:], lhsT=w2_t[:], rhs=pooled[:], start=True, stop=True)
                a_t = per.tile([D, 1], F32)
                nc.scalar.activation(
                    out=a_t[:],
                    in_=ps2[:],
                    func=mybir.ActivationFunctionType.Relu,
                    bias=b2_t[:],
                )
                ps3 = cps.tile([DA, 1], F32, tag="tail")
                nc.tensor.matmul(ps3[:], lhsT=w3_t[:], rhs=a_t[:], start=True, stop=True)
                o_t = per.tile([DA, 1], F32)
                nc.scalar.activation(
                    out=o_t[:],
                    in_=ps3[:],
                    func=mybir.ActivationFunctionType.Tanh,
                    bias=b3_t[:],
                )
                nc.sync.dma_start(out=out_ext[:, :], in_=o_t[:])

    nc.compile()
    return nc


def _wrap16(a):
    """Pack a (multiple-of-128)-length idx vector into the SWDGE int16
    layout PER 4096-token chunk: within each chunk, idx i at
    [i % 16, chunk_col_base + i // 16], replicated into rows 16-31."""
    L = len(a)
    w = np.zeros((128, max(L // 16, 1)), np.int16)
    pos = 0
    CH = CHT * 128
    while pos < L:
        n = min(CH, L - pos)
        seg = a[pos : pos + n].reshape(n // 16, 16).T
        w[0:16, pos // 16 : (pos + n) // 16] = seg
        w[16:32, pos // 16 : (pos + n) // 16] = seg
        pos += n
    return w


def _pack_stream(s_list, d_list, NPC):
    """dst-sorted, window+wide tiled token layout, uniform across cores.

    Returns (fullw, nwide, src_arrays, drf_arrays, drw_arrays)."""
    ncores = len(s_list)
    nwin = NPC // W
    NTO = NPC // 128
    NG = (NTO + 1) // 2
    cnt = np.zeros((ncores, nwin), np.int64)
    srt = []
    for c in range(ncores):
        order = np.argsort(d_list[c], kind="stable")
        ds = d_list[c][order]
        ss = s_list[c][order]
        cnt[c] = np.bincount(ds // W, minlength=nwin)
        srt.append((ss, ds))
    # full tiles per window: per-group coordinate descent on the per-window
    # threshold, minimizing uniform tiles = fulls + wide round-up
    fullw = (cnt // 128).min(axis=0)
    for g0 in range(0, nwin, 4):
        ws = list(range(g0, min(g0 + 4, nwin)))

        def g_tiles(fv):
            rm = np.maximum(cnt[:, ws] - fv[None, :] * 128, 0)
            wid = int(np.max(-(-rm.sum(axis=1) // 128)))
            return int(fv.sum()) + wid

        fv = fullw[ws].copy()
        cost = g_tiles(fv)
        for _ in range(12):
            improved = False
            for i in range(len(ws)):
                for dlt in (1, 2, -1):
                    f2 = fv.copy()
                    f2[i] = max(0, f2[i] + dlt)
                    c2 = g_tiles(f2)
                    if c2 < cost:
                        fv, cost = f2, c2
                        improved = True
            if not improved:
                break
        fullw[ws] = fv
    rem = np.maximum(cnt - fullw[None, :] * 128, 0)
    rem_g = np.add.reduceat(rem, np.arange(0, nwin, 4), axis=1)
    nwide = np.maximum(-(-rem_g // 128), 0).max(axis=0)

    nf, nw = int(fullw.sum()), int(nwide.sum())
    ntiles = nf + nw
    L = ntiles * 128

    # token slot layout: full tiles of each window occupy consecutive slots
    # in processing order (group, window, tile), then the group's wide tiles.
    full_base = np.zeros(nwin, np.int64)    # first token slot of window fulls
    wide_base = np.zeros(NG, np.int64)
    drf_col = np.zeros(nwin, np.int64)      # first drf column of window
    drw_col = np.zeros(NG, np.int64)
    pos = fcol = wcol = 0
    for g in range(NG):
        for wi in range(4):
            wabs = g * 4 + wi
            if wabs >= nwin:
                continue
            full_base[wabs] = pos
            drf_col[wabs] = fcol
            pos += fullw[wabs] * 128
            fcol += fullw[wabs]
        wide_base[g] = pos
        drw_col[g] = wcol
        pos += nwide[g] * 128
        wcol += nwide[g]

    src_arrays, drf_arrays, drw_arrays = [], [], []
    for c in range(ncores):
        ss, ds = srt[c]
        src_tok = np.zeros(L, np.int64)
        dr_tok = np.full(L, -1.0, np.float32)
        if len(ds):
            wins = ds // W
            wstart = np.r_[0, np.cumsum(cnt[c])][wins]
            rank = np.arange(len(ds)) - wstart
            nfull_t = fullw[wins] * 128
            isfull = rank < nfull_t
            posn = np.empty(len(ds), np.int64)
            drv = np.empty(len(ds), np.float32)
            posn[isfull] = full_base[wins[isfull]] + rank[isfull]
            drv[isfull] = (ds[isfull] - wins[isfull] * W).astype(np.float32)
            # wides: per group, leftover tokens packed consecutively
            if (~isfull).any():
                wi = ~isfull
                gs = wins[wi] // 4
                # rank among the group's leftover tokens
                lrank = rank[wi] - nfull_t[wi]
                # offset of this window's leftovers within the group run
                rem_row = rem[c]
                off_in_g = np.zeros(nwin, np.int64)
                for g0 in range(NG):
                    run = 0
                    for wi2 in range(4):
                        wabs = g0 * 4 + wi2
                        if wabs >= nwin:
                            continue
                        off_in_g[wabs] = run
                        run += rem_row[wabs]
                posn[wi] = wide_base[gs] + off_in_g[wins[wi]] + lrank
                drv[wi] = (ds[wi] - gs * GCOL).astype(np.float32)
            src_tok[posn] = ss
            dr_tok[posn] = drv
        src_arrays.append(_wrap16(src_tok.astype(np.int16)))
        # dstrel tables: one bf16 column per tile
        if nf:
            dv = np.full((nf, 128), -1.0, np.float32)
            fi = 0
            for g in range(NG):
                for wi in range(4):
                    wabs = g * 4 + wi
                    if wabs >= nwin:
                        continue
                    for t in range(fullw[wabs]):
                        dv[drf_col[wabs] + t] = dr_tok[
                            full_base[wabs] + t * 128 : full_base[wabs] + (t + 1) * 128
                        ]
            drf_arrays.append(np.ascontiguousarray(dv.astype(NPBF).T))
        else:
            drf_arrays.append(np.zeros((128, 1), NPBF))
        if nw:
            dv = np.full((nw, 128), -1.0, np.float32)
            for g in range(NG):
                for t in range(nwide[g]):
                    dv[drw_col[g] + t] = dr_tok[
                        wide_base[g] + t * 128 : wide_base[g] + (t + 1) * 128
                    ]
            drw_arrays.append(np.ascontiguousarray(dv.astype(NPBF).T))
        else:
            drw_arrays.append(np.zeros((128, 1), NPBF))
    return (
        tuple(int(v) for v in fullw),
        tuple(int(v) for v in nwide),
        src_arrays,
        drf_arrays,
        drw_arrays,
    )


def prep(x, edge_index, conv_w, conv_b, ln_g, ln_b, w2, b2, w3, b3):
    """Host-side sharding. Returns (cfg, in_maps)."""
    x = np.asarray(x, np.float32)
    ei = np.asarray(edge_index).astype(np.int64)
    conv_w = np.asarray(conv_w, np.float32)
    conv_b = np.asarray(conv_b, np.float32)
    ln_g = np.asarray(ln_g, np.float32)
    ln_b = np.asarray(ln_b, np.float32)
    w2 = np.asarray(w2, np.float32)
    b2 = np.asarray(b2, np.float32)
    w3 = np.asarray(w3, np.float32)
    b3 = np.asarray(b3, np.float32)

    N, Din = x.shape
    NPAD = _round_up(N, 1024)
    NPC = NPAD // NCORES
    NTO = NPC // 128

    src, dst = ei[0], ei[1]
    deg = np.bincount(dst, minlength=N).astype(np.float64) + 1.0
    dinv = 1.0 / np.sqrt(deg)

    ys = (x.astype(np.float64) * dinv[:, None]).astype(np.float32)
    xrow = np.zeros((NPAD, Din), NPBF)
    xrow[:N] = ys.astype(NPBF)
    dinv = dinv.astype(np.float32)

    core = dst // NPC
    sA, dA, sB, dB = [], [], [], []
    for c in range(NCORES):
        m = core == c
        s = src[m]
        d = dst[m] - c * NPC
        a = s < HALF
        sA.append(s[a])
        dA.append(d[a])
        sB.append(s[~a] - HALF)
        dB.append(d[~a])

    # Per-core node-tile permutation: rank-order node tiles by edge count so
    # high-count tiles rank-align across cores (tightens the min/max-over-
    # cores uniform tiling).  dinvo/yselfT columns permuted to match.
    perms = []
    for c in range(NCORES):
        tot = np.bincount(dA[c] // 128, minlength=NTO) + np.bincount(
            dB[c] // 128, minlength=NTO
        )
        perm = np.argsort(-tot, kind="stable")  # rank r -> physical tile
        perms.append(perm)
        rank_of = np.empty(NTO, np.int64)
        rank_of[perm] = np.arange(NTO)
        dA[c] = rank_of[dA[c] // 128] * 128 + dA[c] % 128
        dB[c] = rank_of[dB[c] // 128] * 128 + dB[c] % 128

    fullw_a, nwide_a, sa_arr, dfa_arr, dwa_arr = _pack_stream(sA, dA, NPC)
    fullw_b, nwide_b, sb_arr, dfb_arr, dwb_arr = _pack_stream(sB, dB, NPC)

    cfg = {
        "NPAD": NPAD,
        "fullw_a": fullw_a,
        "nwide_a": nwide_a,
        "fullw_b": fullw_b,
        "nwide_b": nwide_b,
    }

    iwf = np.arange(W, dtype=np.float32).astype(NPBF)
    iww = np.arange(GCOL, dtype=np.float32).astype(NPBF)
    in_maps = []
    for c in range(NCORES):
        m = {
            "xa": xrow[: min(HALF, NPAD)],
            "cw": conv_w.astype(NPBF),
            "iwf": iwf,
            "iww": iww,
            "cb": conv_b,
            "w2": (ln_g[:, None] * w2).astype(np.float32),
            "b2": (b2 + (ln_b * float(N)) @ w2).astype(np.float32),
            "w3": w3,
            "b3": b3,
        }
        if NPAD > HALF:
            m["xb"] = xrow[HALF:]
        if sum(fullw_a) + sum(nwide_a):
            m["src0"] = sa_arr[c]
            if sum(fullw_a):
                m["drf0"] = dfa_arr[c]
            if sum(nwide_a):
                m["drw0"] = dwa_arr[c]
        if sum(fullw_b) + sum(nwide_b):
            m["src1"] = sb_arr[c]
            if sum(fullw_b):
                m["drf1"] = dfb_arr[c]
            if sum(nwide_b):
                m["drw1"] = dwb_arr[c]
        dpad = np.zeros(NPC, np.float32)
        ypad = np.zeros((NPC, Din), np.float32)
        cnt = max(0, min((c + 1) * NPC, N) - c * NPC)
        dpad[:cnt] = dinv[c * NPC : c * NPC + cnt]
        ypad[:cnt] = ys[c * NPC : c * NPC + cnt]
        m["dinvo"] = np.ascontiguousarray(dpad.reshape(NTO, 128)[perms[c]].T)
        # own-node y rows, node-tile-permuted, transposed to [feat, NPC]
        m["yselfT"] = np.ascontiguousarray(
            ypad.reshape(NTO, 128, Din)[perms[c]].reshape(NPC, Din).T.astype(NPBF)
        )
        in_maps.append(m)
    return cfg, in_maps


_CACHE = {}


def kernel(**inputs):
    cfg, in_maps = prep(
        inputs["x"],
        inputs["edge_index"],
        inputs["conv_w"],
        inputs["conv_b"],
        inputs["ln_g"],
        inputs["ln_b"],
        inputs["w2"],
        inputs["b2"],
        inputs["w3"],
        inputs["b3"],
    )
    key = (
        cfg["NPAD"],
        cfg["fullw_a"],
        cfg["nwide_a"],
        cfg["fullw_b"],
        cfg["nwide_b"],
    )
    if key not in _CACHE:
        _CACHE[key] = build_graph(cfg)
    nc = _CACHE[key]
    res = run_bass_kernel_spmd(nc, in_maps, core_ids=list(range(NCORES)))
    return np.ascontiguousarray(
        res.results[0]["out"].astype(np.float32).reshape(1, DA)
    )
